# revision 23
# baseline (speedup 1.0000x reference)
"""HGNN layer kernel for 8 TRN2 NeuronCores (Bass/Tile, SPMD row-sharded).

Math (reference):
    dv = H.sum(1); de = H.sum(0)
    Xs = X * dv^-1/2
    M  = H^T @ Xs            [E, F]
    M  = M * de^-1
    Xn = (H @ M) * dv^-1/2   [N, F]
    out = Xn @ W^T + b

Distribution: rows of X/H sharded over 8 cores (N=8192 -> 1024 rows/core).
GEMM1 (H^T @ Xs) is a local partial GEMM; the [E, F] partial plus the
partial column-sum `de` are fused into ONE AllReduce. Everything after
that is row-parallel.

v4 design notes (trace-driven):
- bf16 everywhere that streams from HBM; H is binary so exact in bf16.
- Host prep (same spirit as the host-pretransposed H^T): dvis = dv^-1/2
  and Xs = X * dvis are computed on the host, and a ones column is
  appended -> XSA [NL, 257] bf16. On-chip GEMM1 then depends only on
  raw input loads; no dv chain on the critical path.
- GEMM1 in [e, f] layout: stationary = H chunks [128n, 128e], moving =
  XSA (257 cols). The de column-sum rides along for free. 64 matmuls.
- ONE AllReduce [E, F+1] bf16 (528 KB); its ~25us mesh + ~13us arming
  latency is the irreducible core of this kernel's runtime.
- H is loaded as 8 per-tile DMAs (first tile lands in ~1us so GEMM1
  streams at DMA pace); H^T is one DMA issued AFTER the cc_in write on
  the sync queue, so its 2 MiB moves inside the AllReduce window
  instead of competing with the H/XSA startup loads.
- Post-AR, M'^T comes back via X-bar DMA transpose (no PE transposes).
- Output finalize (x dvis + bias) on vector, written as two bf16 DMAs,
  upcast to f32 on the host. Mw de^-1 scaling runs on the ACT engine
  (scaled copy from PSUM) to keep vector off the GEMM2 critical path.
"""

import os
import sys
import types

import ml_dtypes
import numpy as np

BF_NP = ml_dtypes.bfloat16


def _ensure_axon_hooks_module():
    """bass_utils imports antenv.axon_hooks when tracing; some images
    lack it. Provide a stub (and try to wire the real ctypes hook) so
    trace paths degrade gracefully instead of crashing."""
    try:
        import antenv.axon_hooks  # noqa: F401
        return
    except ImportError:
        pass
    try:
        import antenv
    except ImportError:
        return
    mod = types.ModuleType("antenv.axon_hooks")
    state = {"hook": None}
    mod.get_axon_ntff_profile_hook = lambda: state["hook"]
    mod.set_axon_ntff_profile_hook = lambda h: state.__setitem__("hook", h)
    sys.modules["antenv.axon_hooks"] = mod
    antenv.axon_hooks = mod
    try:
        from trn_agent_boot.trn_boot import _ntff_profile_via_ctypes
        hook = _ntff_profile_via_ctypes("/opt/axon/libaxon_pjrt.so")
        if hook is not None:
            state["hook"] = hook
    except Exception:
        pass


_ensure_axon_hooks_module()

N, E, F = 8192, 1024, 256
P = 128
NC_COUNT = 8
NL = N // NC_COUNT          # 1024 rows per core
NT = NL // P                # 8 row tiles per core
ET = E // P                 # 8 e-chunks
FI = F // P                 # 2 fi-chunks
FA = F + 1                  # moving width with the fused-de ones column

_cache = {}


def _build():
    from concourse import bacc, bass, tile, mybir

    f32 = mybir.dt.float32
    bf = mybir.dt.bfloat16

    nc = bacc.Bacc("TRN2", target_bir_lowering=False, debug=False,
                   num_devices=NC_COUNT)

    XSA_d = nc.dram_tensor("XSA", [NL, FA], bf, kind="ExternalInput")
    H_d = nc.dram_tensor("H", [NL, E], bf, kind="ExternalInput")
    HT_d = nc.dram_tensor("HT", [E, NL], bf, kind="ExternalInput")
    WT_d = nc.dram_tensor("WT", [F, F], bf, kind="ExternalInput")
    B_d = nc.dram_tensor("bias", [P, F], f32, kind="ExternalInput")
    DVIS_d = nc.dram_tensor("dvis", [P, NT], f32, kind="ExternalInput")
    out_d = nc.dram_tensor("out", [NL, F], bf, kind="ExternalOutput")

    with tile.TileContext(nc) as tc:
        with (
            tc.tile_pool(name="const", bufs=1) as constp,
            tc.tile_pool(name="hp", bufs=1) as hp,
            tc.tile_pool(name="htp", bufs=1) as htp,
            tc.tile_pool(name="xp", bufs=1) as xp,
            tc.tile_pool(name="sp", bufs=1) as sp,
            tc.tile_pool(name="mwp", bufs=1) as mwp,
            tc.tile_pool(name="ps_mt", bufs=3, space="PSUM") as ps_mt,
            tc.tile_pool(name="ps_b", bufs=3, space="PSUM") as ps_b,
            tc.tile_pool(name="dram", bufs=1, space="DRAM") as dramp,
        ):
            # ---- H per-tile on the sync queue: first tile lands fast and
            # GEMM1 streams behind the loads.
            h = []
            for i in range(NT):
                hi = hp.tile([P, E], bf, name=f"h{i}")
                nc.sync.dma_start(hi[:], H_d[i * P:(i + 1) * P, :])
                h.append(hi)

            # XSA / W / bias / dvis on the scalar HWDGE queue. XSA is
            # per-tile like H so GEMM1's first group isn't gated on the
            # whole 0.5 MiB load.
            xsa = []
            for i in range(NT):
                xi = xp.tile([P, FA], bf, name=f"xsa{i}")
                nc.scalar.dma_start(xi[:], XSA_d[i * P:(i + 1) * P, :])
                xsa.append(xi)
            wt_big = constp.tile([P, FI * F], bf)
            nc.scalar.dma_start(
                wt_big[:].rearrange("p (c f) -> p c f", c=FI),
                WT_d[:, :].rearrange("(c p) f -> p c f", p=P))
            bias = constp.tile([P, F], f32)
            nc.scalar.dma_start(bias[:], B_d[:, :])
            dvis = constp.tile([P, NT], f32)
            nc.scalar.dma_start(dvis[:], DVIS_d[:, :])

            # ---- collective bounce buffers: [E, F+1] bf16 ----
            cc_in = dramp.tile([E, FA], bf, name="cc_in")
            cc_out = dramp.tile([E, FA], bf, name="cc_out",
                                addr_space="Shared")

            # ---- GEMM1 (+fused de): psum[e, f'] = sum_n H[n,e] [Xs|1][n,f']
            m_big = mwp.tile([P, ET * FA], bf)
            for j in range(ET):
                mt_ps = ps_mt.tile([P, FA], f32, name="mt_ps")
                for i in range(NT):
                    nc.tensor.matmul(
                        mt_ps[:],
                        h[i][:, j * P:(j + 1) * P],
                        xsa[i][:],
                        start=(i == 0), stop=(i == NT - 1),
                    )
                if j % 2 == 0:
                    nc.vector.tensor_copy(m_big[:, j * FA:(j + 1) * FA],
                                          mt_ps[:])
                else:
                    nc.scalar.copy(m_big[:, j * FA:(j + 1) * FA], mt_ps[:])

            nc.sync.dma_start(
                cc_in[:, :].rearrange("(j p) f -> p j f", p=P),
                m_big[:].rearrange("p (j f) -> p j f", j=ET))

            # ---- AllReduce of [M | de] over all 8 cores ----
            nc.gpsimd.collective_compute(
                "AllReduce",
                mybir.AluOpType.add,
                replica_groups=[list(range(NC_COUNT))],
                ins=[cc_in[:].opt()],
                outs=[cc_out[:].opt()],
            )

            # ---- H^T load inside the AllReduce window (sync engine is
            # stalled on the cc_in write completion just before this, so
            # the 2 MiB moves while the mesh arms).
            ht_big = htp.tile([P, ET * NL], bf)
            nc.sync.dma_start(
                ht_big[:].rearrange("p (j n) -> p j n", j=ET),
                HT_d[:, :].rearrange("(j p) n -> p j n", p=P))

            # ---- read back: M'^T fi-chunks via X-bar DMA transpose, and
            # the de column reshaped to [128, 8] ----
            mtin = []
            for c in range(FI):
                mc = mwp.tile([P, E], bf, name=f"mtin{c}")
                nc.sync.dma_start(mc[:], cc_out[:, c * P:(c + 1) * P],
                                  transpose=True)
                mtin.append(mc)
            de_sb = sp.tile([P, ET], bf)
            nc.scalar.dma_start(
                de_sb[:].rearrange("p (j o) -> p j o", o=1),
                cc_out[:, F:FA].rearrange("(j p) o -> p j o", p=P))
            de_inv = sp.tile([P, ET], f32)
            nc.vector.reciprocal(de_inv[:], de_sb[:])

            # ---- GEMM-W: Mw[e, fo] = sum_fi M'[e, fi] W^T[fi, fo]; x de^-1
            mw = []
            for j in range(ET):
                mw_ps = ps_b.tile([P, F], f32, name="mw_ps", tag="ps_post")
                for c in range(FI):
                    nc.tensor.matmul(
                        mw_ps[:],
                        mtin[c][:, j * P:(j + 1) * P],
                        wt_big[:, c * F:(c + 1) * F],
                        start=(c == 0), stop=(c == FI - 1),
                    )
                mwj = mwp.tile([P, F], bf, name=f"mw{j}")
                nc.scalar.mul(mwj[:], mw_ps[:], de_inv[:, j:j + 1])
                mw.append(mwj)

            # ---- GEMM2: out[n, fo] = (sum_e H^T[e,n] Mw[e,fo]) * dv^-1/2 + b
            o_big = mwp.tile([P, NT * F], bf)
            for jn in range(NT):
                o_ps = ps_b.tile([P, F], f32, name="o_ps", tag="ps_post")
                for j in range(ET):
                    nc.tensor.matmul(
                        o_ps[:],
                        ht_big[:, j * NL + jn * P:j * NL + (jn + 1) * P],
                        mw[j][:],
                        start=(j == 0), stop=(j == ET - 1),
                    )
                nc.vector.scalar_tensor_tensor(
                    o_big[:, jn * F:(jn + 1) * F], o_ps[:],
                    dvis[:, jn:jn + 1], bias[:],
                    op0=mybir.AluOpType.mult, op1=mybir.AluOpType.add)
            half = NT // 2
            nc.sync.dma_start(
                out_d[0:half * P, :].rearrange("(i p) f -> p i f", p=P),
                o_big[:, 0:half * F].rearrange("p (i f) -> p i f", i=half))
            nc.sync.dma_start(
                out_d[half * P:NL, :].rearrange("(i p) f -> p i f", p=P),
                o_big[:, half * F:].rearrange("p (i f) -> p i f", i=half))

    nc.compile()
    return nc


def _get_nc():
    if "nc" not in _cache:
        _cache["nc"] = _build()
    return _cache["nc"]


def kernel(X, H, W, b):
    from concourse import bass_utils

    nc = _get_nc()

    X = np.asarray(X, dtype=np.float32)
    H = np.asarray(H, dtype=np.float32)
    W = np.asarray(W, dtype=np.float32)
    b = np.asarray(b, dtype=np.float32)

    WT = np.ascontiguousarray(W.T.astype(BF_NP))
    bias = np.ascontiguousarray(np.tile(b[None, :], (P, 1)))

    dvis_full = 1.0 / np.sqrt(H.sum(axis=1))          # [N] f32
    XSA = np.empty((N, FA), dtype=BF_NP)
    XSA[:, :F] = (X * dvis_full[:, None]).astype(BF_NP)
    XSA[:, F] = np.float32(1.0)

    Hb = H.astype(BF_NP)

    in_maps = []
    for c in range(NC_COUNT):
        sl = slice(c * NL, (c + 1) * NL)
        Hc = np.ascontiguousarray(Hb[sl])
        dv_c = np.ascontiguousarray(
            dvis_full[sl].reshape(NT, P).T.astype(np.float32))
        in_maps.append({
            "XSA": np.ascontiguousarray(XSA[sl]),
            "H": Hc,
            "HT": np.ascontiguousarray(Hc.T),
            "WT": WT,
            "bias": bias,
            "dvis": dv_c,
        })

    res = bass_utils.run_bass_kernel_spmd(
        nc, in_maps, core_ids=list(range(NC_COUNT)),
        trace=bool(int(os.environ.get("HGNN_TRACE", "0"))),
    )
    _cache["last_result"] = res
    out = np.concatenate(
        [np.asarray(res.results[c]["out"]).astype(np.float32)
         for c in range(NC_COUNT)], axis=0)
    return out


# revision 24
# speedup vs baseline: 1.0268x; 1.0268x over previous
"""HGNN layer kernel for 8 TRN2 NeuronCores (Bass/Tile, SPMD row-sharded).

Math (reference):
    dv = H.sum(1); de = H.sum(0)
    Xs = X * dv^-1/2
    M  = H^T @ Xs            [E, F]
    M  = M * de^-1
    Xn = (H @ M) * dv^-1/2   [N, F]
    out = Xn @ W^T + b

Distribution: rows of X/H sharded over 8 cores (N=8192 -> 1024 rows/core).
GEMM1 (H^T @ Xs) is a local partial GEMM; the [E, F] partial plus the
partial column-sum `de` are fused into ONE AllReduce. Everything after
that is row-parallel.

v4 design notes (trace-driven):
- bf16 everywhere that streams from HBM; H is binary so exact in bf16.
- Host prep (same spirit as the host-pretransposed H^T): dvis = dv^-1/2
  and Xs = X * dvis are computed on the host, and a ones column is
  appended -> XSA [NL, 257] bf16. On-chip GEMM1 then depends only on
  raw input loads; no dv chain on the critical path.
- GEMM1 in [e, f] layout: stationary = H chunks [128n, 128e], moving =
  XSA (257 cols). The de column-sum rides along for free. 64 matmuls.
- ONE AllReduce [E, F+1] bf16 (528 KB); its ~25us mesh + ~13us arming
  latency is the irreducible core of this kernel's runtime.
- H is loaded as 8 per-tile DMAs (first tile lands in ~1us so GEMM1
  streams at DMA pace); H^T is one DMA issued AFTER the cc_in write on
  the sync queue, so its 2 MiB moves inside the AllReduce window
  instead of competing with the H/XSA startup loads.
- Post-AR, M'^T comes back via X-bar DMA transpose (no PE transposes).
- Output finalize (x dvis + bias) on vector, written as two bf16 DMAs,
  upcast to f32 on the host. Mw de^-1 scaling runs on the ACT engine
  (scaled copy from PSUM) to keep vector off the GEMM2 critical path.
"""

import os
import sys
import types

import ml_dtypes
import numpy as np

BF_NP = ml_dtypes.bfloat16


def _ensure_axon_hooks_module():
    """bass_utils imports antenv.axon_hooks when tracing; some images
    lack it. Provide a stub (and try to wire the real ctypes hook) so
    trace paths degrade gracefully instead of crashing."""
    try:
        import antenv.axon_hooks  # noqa: F401
        return
    except ImportError:
        pass
    try:
        import antenv
    except ImportError:
        return
    mod = types.ModuleType("antenv.axon_hooks")
    state = {"hook": None}
    mod.get_axon_ntff_profile_hook = lambda: state["hook"]
    mod.set_axon_ntff_profile_hook = lambda h: state.__setitem__("hook", h)
    sys.modules["antenv.axon_hooks"] = mod
    antenv.axon_hooks = mod
    try:
        from trn_agent_boot.trn_boot import _ntff_profile_via_ctypes
        hook = _ntff_profile_via_ctypes("/opt/axon/libaxon_pjrt.so")
        if hook is not None:
            state["hook"] = hook
    except Exception:
        pass


_ensure_axon_hooks_module()

N, E, F = 8192, 1024, 256
P = 128
NC_COUNT = 8
NL = N // NC_COUNT          # 1024 rows per core
NT = NL // P                # 8 row tiles per core
ET = E // P                 # 8 e-chunks
FI = F // P                 # 2 fi-chunks
FA = F + 1                  # moving width with the fused-de ones column

_cache = {}


def _build():
    from concourse import bacc, bass, tile, mybir

    f32 = mybir.dt.float32
    bf = mybir.dt.bfloat16

    nc = bacc.Bacc("TRN2", target_bir_lowering=False, debug=False,
                   num_devices=NC_COUNT)

    XSA_d = nc.dram_tensor("XSA", [NL, FA], bf, kind="ExternalInput")
    H_d = nc.dram_tensor("H", [NL, E], bf, kind="ExternalInput")
    HT_d = nc.dram_tensor("HT", [E, NL], bf, kind="ExternalInput")
    WT_d = nc.dram_tensor("WT", [F, F], bf, kind="ExternalInput")
    B_d = nc.dram_tensor("bias", [P, F], f32, kind="ExternalInput")
    DVIS_d = nc.dram_tensor("dvis", [P, NT], f32, kind="ExternalInput")
    out_d = nc.dram_tensor("out", [NL, F], bf, kind="ExternalOutput")

    with tile.TileContext(nc) as tc:
        with (
            tc.tile_pool(name="const", bufs=1) as constp,
            tc.tile_pool(name="hp", bufs=1) as hp,
            tc.tile_pool(name="htp", bufs=1) as htp,
            tc.tile_pool(name="xp", bufs=1) as xp,
            tc.tile_pool(name="sp", bufs=1) as sp,
            tc.tile_pool(name="mwp", bufs=1) as mwp,
            tc.tile_pool(name="ps_mt", bufs=3, space="PSUM") as ps_mt,
            tc.tile_pool(name="ps_b", bufs=3, space="PSUM") as ps_b,
            tc.tile_pool(name="dram", bufs=1, space="DRAM") as dramp,
        ):
            # ---- H per-tile on the sync queue: first tile lands fast and
            # GEMM1 streams behind the loads.
            h = []
            for i in range(NT):
                hi = hp.tile([P, E], bf, name=f"h{i}")
                nc.sync.dma_start(hi[:], H_d[i * P:(i + 1) * P, :])
                h.append(hi)

            # XSA / W / bias / dvis on the scalar HWDGE queue.
            xsa = xp.tile([P, NT * FA], bf)
            nc.scalar.dma_start(
                xsa[:].rearrange("p (i f) -> p i f", i=NT),
                XSA_d[:, :].rearrange("(i p) f -> p i f", p=P))
            wt_big = constp.tile([P, FI * F], bf)
            nc.scalar.dma_start(
                wt_big[:].rearrange("p (c f) -> p c f", c=FI),
                WT_d[:, :].rearrange("(c p) f -> p c f", p=P))
            bias = constp.tile([P, F], f32)
            nc.scalar.dma_start(bias[:], B_d[:, :])
            dvis = constp.tile([P, NT], f32)
            nc.scalar.dma_start(dvis[:], DVIS_d[:, :])

            # ---- collective bounce buffers: [E, F+1] bf16 ----
            cc_in = dramp.tile([E, FA], bf, name="cc_in")
            cc_out = dramp.tile([E, FA], bf, name="cc_out",
                                addr_space="Shared")

            # ---- GEMM1 (+fused de): psum[e, f'] = sum_n H[n,e] [Xs|1][n,f']
            m_big = mwp.tile([P, ET * FA], bf)
            for j in range(ET):
                mt_ps = ps_mt.tile([P, FA], f32, name="mt_ps")
                for i in range(NT):
                    nc.tensor.matmul(
                        mt_ps[:],
                        h[i][:, j * P:(j + 1) * P],
                        xsa[:, i * FA:(i + 1) * FA],
                        start=(i == 0), stop=(i == NT - 1),
                    )
                if j % 2 == 0:
                    nc.vector.tensor_copy(m_big[:, j * FA:(j + 1) * FA],
                                          mt_ps[:])
                else:
                    nc.scalar.copy(m_big[:, j * FA:(j + 1) * FA], mt_ps[:])

            nc.sync.dma_start(
                cc_in[:, :].rearrange("(j p) f -> p j f", p=P),
                m_big[:].rearrange("p (j f) -> p j f", j=ET))

            # ---- AllReduce of [M | de] over all 8 cores ----
            nc.gpsimd.collective_compute(
                "AllReduce",
                mybir.AluOpType.add,
                replica_groups=[list(range(NC_COUNT))],
                ins=[cc_in[:].opt()],
                outs=[cc_out[:].opt()],
            )

            # ---- H^T load inside the AllReduce window (sync engine is
            # stalled on the cc_in write completion just before this, so
            # the 2 MiB moves while the mesh arms).
            ht_big = htp.tile([P, ET * NL], bf)
            nc.sync.dma_start(
                ht_big[:].rearrange("p (j n) -> p j n", j=ET),
                HT_d[:, :].rearrange("(j p) n -> p j n", p=P))

            # ---- read back: M'^T fi-chunks via X-bar DMA transpose, and
            # the de column reshaped to [128, 8] ----
            mtin = []
            for c in range(FI):
                mc = mwp.tile([P, E], bf, name=f"mtin{c}")
                nc.sync.dma_start(mc[:], cc_out[:, c * P:(c + 1) * P],
                                  transpose=True)
                mtin.append(mc)
            de_sb = sp.tile([P, ET], bf)
            nc.scalar.dma_start(
                de_sb[:].rearrange("p (j o) -> p j o", o=1),
                cc_out[:, F:FA].rearrange("(j p) o -> p j o", p=P))
            de_inv = sp.tile([P, ET], f32)
            nc.vector.reciprocal(de_inv[:], de_sb[:])

            # ---- GEMM-W: Mw[e, fo] = sum_fi M'[e, fi] W^T[fi, fo]; x de^-1
            mw = []
            for j in range(ET):
                mw_ps = ps_b.tile([P, F], f32, name="mw_ps", tag="ps_post")
                for c in range(FI):
                    nc.tensor.matmul(
                        mw_ps[:],
                        mtin[c][:, j * P:(j + 1) * P],
                        wt_big[:, c * F:(c + 1) * F],
                        start=(c == 0), stop=(c == FI - 1),
                    )
                mwj = mwp.tile([P, F], bf, name=f"mw{j}")
                nc.scalar.mul(mwj[:], mw_ps[:], de_inv[:, j:j + 1])
                mw.append(mwj)

            # ---- GEMM2: out[n, fo] = (sum_e H^T[e,n] Mw[e,fo]) * dv^-1/2 + b
            o_big = mwp.tile([P, NT * F], bf)
            for jn in range(NT):
                o_ps = ps_b.tile([P, F], f32, name="o_ps", tag="ps_post")
                for j in range(ET):
                    nc.tensor.matmul(
                        o_ps[:],
                        ht_big[:, j * NL + jn * P:j * NL + (jn + 1) * P],
                        mw[j][:],
                        start=(j == 0), stop=(j == ET - 1),
                    )
                nc.vector.scalar_tensor_tensor(
                    o_big[:, jn * F:(jn + 1) * F], o_ps[:],
                    dvis[:, jn:jn + 1], bias[:],
                    op0=mybir.AluOpType.mult, op1=mybir.AluOpType.add)
            half = NT // 2
            nc.sync.dma_start(
                out_d[0:half * P, :].rearrange("(i p) f -> p i f", p=P),
                o_big[:, 0:half * F].rearrange("p (i f) -> p i f", i=half))
            nc.sync.dma_start(
                out_d[half * P:NL, :].rearrange("(i p) f -> p i f", p=P),
                o_big[:, half * F:].rearrange("p (i f) -> p i f", i=half))

    nc.compile()
    return nc


def _get_nc():
    if "nc" not in _cache:
        _cache["nc"] = _build()
    return _cache["nc"]


def kernel(X, H, W, b):
    from concourse import bass_utils

    nc = _get_nc()

    X = np.asarray(X, dtype=np.float32)
    H = np.asarray(H, dtype=np.float32)
    W = np.asarray(W, dtype=np.float32)
    b = np.asarray(b, dtype=np.float32)

    WT = np.ascontiguousarray(W.T.astype(BF_NP))
    bias = np.ascontiguousarray(np.tile(b[None, :], (P, 1)))

    dvis_full = 1.0 / np.sqrt(H.sum(axis=1))          # [N] f32
    XSA = np.empty((N, FA), dtype=BF_NP)
    XSA[:, :F] = (X * dvis_full[:, None]).astype(BF_NP)
    XSA[:, F] = np.float32(1.0)

    Hb = H.astype(BF_NP)

    in_maps = []
    for c in range(NC_COUNT):
        sl = slice(c * NL, (c + 1) * NL)
        Hc = np.ascontiguousarray(Hb[sl])
        dv_c = np.ascontiguousarray(
            dvis_full[sl].reshape(NT, P).T.astype(np.float32))
        in_maps.append({
            "XSA": np.ascontiguousarray(XSA[sl]),
            "H": Hc,
            "HT": np.ascontiguousarray(Hc.T),
            "WT": WT,
            "bias": bias,
            "dvis": dv_c,
        })

    res = bass_utils.run_bass_kernel_spmd(
        nc, in_maps, core_ids=list(range(NC_COUNT)),
        trace=bool(int(os.environ.get("HGNN_TRACE", "0"))),
    )
    _cache["last_result"] = res
    out = np.concatenate(
        [np.asarray(res.results[c]["out"]).astype(np.float32)
         for c in range(NC_COUNT)], axis=0)
    return out
